# revision 19
# baseline (speedup 1.0000x reference)
"""Multi-head attention (B=2, S=2048, D=1024, H=16, causal) on 8 TRN2 NeuronCores.

Sharding: core c -> (batch b = c//4, head-group g = c%4, heads 4g..4g+3).
Each core computes Q/K/V projections for its 4 heads, causal attention, and
a partial output projection (its 256 d_model columns of ctx @ wo.T).  Host
sums the 4 partials per batch and adds bo.

v2 restructure vs baseline: the kernel is ACT(exp)-limited (~92us of exp),
so everything is scheduled to hide under the exp stream:
 - inputs DMA'd in s-quarter slabs so attention pair 0 starts ~17us in
   (f-tile-0 q/k projections + v quarter 0 only gate the first q-chunk);
 - f-tile-1 projections, remaining v projection, and output-projection
   matmuls are "filler" units pumped into PE slack inside the attention
   k-tile loop (PE ~0.65us/kt vs ACT 1.15us/kt);
 - softmax denominators via DVE reciprocal_approx_fast + DMA partition
   broadcast (pipelined per q-chunk; PE ones-matmul broadcast on the last
   chunk to cut the tail);
 - q/k projection drains on DVE (tensor_scalar w/ bias), output copies
   split DVE/ACT so ACT does (almost) only exp.
PSUM: sc tag [128,1024] bufs=2 (scores double-buffer) + pc tag [128,1024]
bufs=2 (ctx accumulator + one flex slot for proj/v/out-proj groups).
"""

import sys
from collections import deque

for _p in ("/opt/trn_rl_repo",):
    if _p not in sys.path:
        sys.path.insert(0, _p)

import ml_dtypes
import numpy as np

import concourse.bass as bass
import concourse.mybir as mybir
import concourse.tile as tile
from concourse import bacc, bass_utils

F32 = mybir.dt.float32
BF16 = mybir.dt.bfloat16
AF = mybir.ActivationFunctionType
ALU = mybir.AluOpType

N_CORES = 8
B, S, D, H = 2, 2048, 1024, 16
HG = 4              # heads per core
DK = 64             # head dim
F = HG * DK         # 256 features per core
FA = HG * (DK + 1)  # 260: v features + per-head denominator ones-column
DT = D // 128       # 8 d-chunks
FT = F // 128       # 2 f-tiles (head pairs)
SQ = 4              # s-quarters of 512


def _build():
    nc = bacc.Bacc("TRN2", target_bir_lowering=False, debug=False,
                   num_devices=N_CORES)

    def din(name, shape, dt=F32):
        return nc.dram_tensor(name, shape, dt, kind="ExternalInput").ap()

    # host pre-arranges everything partition-major so every DMA is a flat
    # 2D slab (3D fan-out DMAs race on their shared completion semaphore)
    xqT = din("xqT", (SQ * 128, DT * 512), BF16)  # [sq*128+p, d*512+s]
    xkT = din("xkT", (SQ * 128, DT * 512), BF16)
    xvT = din("xvT", (SQ * 128, DT * 512), BF16)
    wqT = din("wqT", (128, DT * F), BF16)         # [p, d*256+f]
    wkT = din("wkT", (128, DT * F), BF16)
    wvT = din("wvT", (128, DT * FA), BF16)        # interleaved ones-columns
    woT = din("woT", (128, FT * D), BF16)         # [p, t*1024+e]
    bq8 = din("bq8", (128, FT))
    bk = din("bk", (128, FT))
    bv260 = din("bv260", (128, FA))  # bv broadcast, 1.0 at ones-columns
    outT = nc.dram_tensor("outT", (D, S), BF16, kind="ExternalOutput").ap()

    with tile.TileContext(nc) as tc:
        with (
            tc.tile_pool(name="const", bufs=1) as cp,
            tc.tile_pool(name="data", bufs=1) as dp,
            tc.tile_pool(name="io", bufs=1) as iop,
            tc.tile_pool(name="dr", bufs=1, space="DRAM") as drp,
            tc.tile_pool(name="pp", bufs=2, space="PSUM") as pp,
        ):
            def psum_sc():
                return pp.tile([128, 1024], F32, name="sc", bufs=2)

            def psum_pc():
                return pp.tile([128, 1024], F32, name="pc", bufs=2)

            # ---- input DMA issues, s-quarter slabs, attn-gating first ----
            xq_t = [dp.tile([128, DT * 512], BF16, name=f"xq{sq}")
                    for sq in range(SQ)]
            xk_t = [dp.tile([128, DT * 512], BF16, name=f"xk{sq}")
                    for sq in range(SQ)]
            xv_t = [dp.tile([128, DT * 512], BF16, name=f"xv{sq}")
                    for sq in range(SQ)]
            for sq in range(SQ):
                for xt, src in ((xq_t, xqT), (xk_t, xkT), (xv_t, xvT)):
                    nc.sync.dma_start(
                        xt[sq][:], src[sq * 128:(sq + 1) * 128, :])

            # ---- weights (sync queue; ACT hwdge fan-out races, inc-6505) --
            wq_sb = cp.tile([128, DT * F], BF16, name="wq")
            wk_sb = cp.tile([128, DT * F], BF16, name="wk")
            wv_sb = cp.tile([128, DT * FA], BF16, name="wv")
            wo_sb = cp.tile([128, FT * D], BF16, name="wo")
            nc.sync.dma_start(wq_sb[:], wqT[:])
            nc.sync.dma_start(wk_sb[:], wkT[:])
            nc.sync.dma_start(wv_sb[:], wvT[:])
            nc.sync.dma_start(wo_sb[:], woT[:])
            bq8_t = cp.tile([128, FT], F32, name="bq8")
            bk_t = cp.tile([128, FT], F32, name="bkt")
            bv_t = cp.tile([128, FA], F32, name="bv")
            nc.sync.dma_start(bq8_t[:], bq8[:])
            nc.sync.dma_start(bk_t[:], bk[:])
            nc.sync.dma_start(bv_t[:], bv260[:])

            # ---- PE warmup (HAM un-throttle) on a DVE-memset tile --------
            wmup = cp.tile([128, 512], BF16, name="wmup")
            nc.vector.memset(wmup[:], 0.0)
            pw = psum_sc()
            for i in range(12):
                nc.tensor.matmul(pw[:, 0:512], wmup[:, 0:128], wmup[:],
                                 start=(i == 0), stop=True,
                                 skip_group_check=True)

            # tri01[p, y] = 1 if y >= p else 0 (diagonal-block causal mask)
            tri = cp.tile([128, 128], BF16, name="tri")
            nc.gpsimd.memset(tri[:], 1.0)
            nc.gpsimd.affine_select(
                out=tri[:], in_=tri[:], compare_op=ALU.is_ge,
                fill=0.0, base=0, pattern=[[1, 128]], channel_multiplier=-1)
            # absorb gpsimd tensor_tensor dispatch warmup off critical path
            gwarm = cp.tile([128, 128], BF16, name="gwarm")
            nc.gpsimd.tensor_tensor(gwarm[:], tri[:], tri[:], op=ALU.mult)

            # ---- persistent per-core tensors -----------------------------
            # qpT/kpT[t][sp]: [f 128, s 1024];  ctxn[t][sp] same layout
            qpT = [[dp.tile([128, 1024], BF16, name=f"qpT{t}{sp}")
                    for sp in range(2)] for t in range(FT)]
            kpT = [[dp.tile([128, 1024], BF16, name=f"kpT{t}{sp}")
                    for sp in range(2)] for t in range(FT)]
            ctxn = [[dp.tile([128, 1024], BF16, name=f"ctxn{t}{sp}")
                     for sp in range(2)] for t in range(FT)]
            vp = [dp.tile([128, FA], BF16, name=f"vp{st}") for st in range(16)]

            # ---- building blocks -----------------------------------------
            def qk_group(sq, t):
                """f-tile t of q and k projections for s-quarter sq.
                One psum slot: q in cols 0:512, k in 512:1024."""
                ps = psum_pc() if t == 1 else psum_sc()
                for d in range(DT):
                    nc.tensor.matmul(
                        ps[:, 0:512],
                        wq_sb[:, d * F + t * 128: d * F + (t + 1) * 128],
                        xq_t[sq][:, d * 512:(d + 1) * 512],
                        start=(d == 0), stop=(d == DT - 1))
                for d in range(DT):
                    nc.tensor.matmul(
                        ps[:, 512:1024],
                        wk_sb[:, d * F + t * 128: d * F + (t + 1) * 128],
                        xk_t[sq][:, d * 512:(d + 1) * 512],
                        start=(d == 0), stop=(d == DT - 1))
                sp, half = sq // 2, sq % 2
                nc.vector.tensor_scalar(
                    qpT[t][sp][:, half * 512:(half + 1) * 512],
                    ps[:, 0:512], 0.125, bq8_t[:, t:t + 1],
                    op0=ALU.mult, op1=ALU.add)
                nc.vector.tensor_scalar_add(
                    kpT[t][sp][:, half * 512:(half + 1) * 512],
                    ps[:, 512:1024], bk_t[:, t:t + 1])

            def v_gen(s8_pair):
                """v projection for two s8 tiles (128 seq positions each)."""
                ps = psum_pc()
                for d in range(DT):
                    for i, s8 in enumerate(s8_pair):
                        sq, s8i = s8 // 4, s8 % 4
                        nc.tensor.matmul(
                            ps[:, i * 512:i * 512 + FA],
                            xv_t[sq][:, d * 512 + s8i * 128:
                                     d * 512 + (s8i + 1) * 128],
                            wv_sb[:, d * FA:(d + 1) * FA],
                            start=(d == 0), stop=(d == DT - 1))
                for i, s8 in enumerate(s8_pair):
                    nc.vector.tensor_add(
                        vp[s8][:], ps[:, i * 512:i * 512 + FA], bv_t[:])

            def outproj_group(e, sp, sh, ob_act=False):
                """out rows e*128.., s-cols sp*1024+sh*512 .. +512."""
                ps = psum_pc()
                for t in range(FT):
                    nc.tensor.matmul(
                        ps[:, 0:512],
                        wo_sb[:, t * D + e * 128: t * D + (e + 1) * 128],
                        ctxn[t][sp][:, sh * 512:(sh + 1) * 512],
                        start=(t == 0), stop=(t == FT - 1))
                ob = iop.tile([128, 512], BF16, name="ob", bufs=4)
                if ob_act:
                    nc.scalar.copy(ob[:], ps[:, 0:512])
                else:
                    nc.vector.tensor_copy(ob[:], ps[:, 0:512])
                nc.sync.dma_start(
                    outT[e * 128:(e + 1) * 128,
                         sp * 1024 + sh * 512: sp * 1024 + (sh + 1) * 512],
                    ob[:])

            # ---- filler pump: PE work hidden in ACT-limited attn windows.
            # Tile deps are trace-order based (a read traced before its
            # producer's write gets NO dependency), so attn_j must require()
            # its producers before tracing consumers; pump() spends leftover
            # slack on whatever is next.
            filler = deque()   # (key, cost_us, emit_fn)
            _budget = [0.0]

            def pump(budget_us):
                _budget[0] += budget_us
                while filler and filler[0][1] <= _budget[0]:
                    key, cost, fn = filler.popleft()
                    fn()
                    _budget[0] -= cost

            def require(keys):
                keep, run = deque(), []
                for item in filler:
                    (run if item[0] in keys else keep).append(item)
                filler.clear()
                filler.extend(keep)
                for _, _, fn in run:
                    fn()

            def flush():
                while filler:
                    filler.popleft()[2]()
                _budget[0] = 0.0

            # ---- attention -----------------------------------------------
            def attn_j(hp, j):
                t = hp
                require({("qk", hp, s) for s in range(j + 1)}
                        | {("v", s) for s in range(j + 1)})
                q0 = j * 512
                sp, half = j // 2, j % 2
                pc = psum_pc()            # ctx: h0 in 0:512, h1 in 512:1024
                started = [False, False]
                kts = list(range(4 * j + 4))  # ascending: ctx group starts full-width

                def ctx_mm(kt, ex, c0, last):
                    w = 512 - c0
                    for hh in range(2):
                        nc.tensor.matmul(
                            pc[0:65, hh * 512 + c0: hh * 512 + 512],
                            vp[kt][:, 65 * (2 * hp + hh):
                                   65 * (2 * hp + hh) + 65],
                            ex[:, hh * 512: hh * 512 + w],
                            start=not started[hh], stop=last)
                        started[hh] = True

                pending = []
                for n, kt in enumerate(kts):
                    c0 = max(0, 128 * kt - q0)
                    w = 512 - c0
                    psc = psum_sc()
                    for hh, off in ((0, 0), (1, 64)):
                        nc.tensor.matmul(
                            psc[:, hh * 512: hh * 512 + w],
                            kpT[t][kt // 8][off:off + 64,
                                            (kt % 8) * 128:(kt % 8 + 1) * 128],
                            qpT[t][sp][off:off + 64,
                                       half * 512 + c0: half * 512 + 512],
                            start=True, stop=True)
                    ex = iop.tile([128, 1024], BF16, name="ex", bufs=6)
                    if w == 512:
                        nc.scalar.activation(ex[:], psc[:], AF.Exp)
                    else:
                        v2 = psc[:].rearrange("p (b c) -> p b c",
                                              c=512)[:, :, 0:w]
                        e2 = ex[:].rearrange("p (b c) -> p b c",
                                             c=512)[:, :, 0:w]
                        nc.scalar.activation(e2, v2, AF.Exp)
                    if 128 * kt >= q0:   # diagonal block: triangular mask
                        nc.gpsimd.tensor_tensor(
                            ex[:, 0:128], ex[:, 0:128], tri[:], op=ALU.mult)
                        nc.gpsimd.tensor_tensor(
                            ex[:, 512:640], ex[:, 512:640], tri[:],
                            op=ALU.mult)
                    pending.append((kt, ex, c0))
                    if len(pending) > 1:
                        ctx_mm(*pending.pop(0), last=False)
                    pump(0.5)
                while pending:
                    ctx_mm(*pending.pop(0), last=(len(pending) == 0))

                # normalize: rec = approx 1/den, broadcast, multiply
                rec = iop.tile([1, 1024], F32, name="rec", bufs=3)
                nc.vector.reciprocal(rec[:], pc[64:65, :])
                dstage = drp.tile([1, 1024], F32, name="dstage", bufs=2)
                nc.sync.dma_start(dstage[:], rec[:])
                bc = iop.tile([128, 1024], F32, name="bc", bufs=3)
                nc.sync.dma_start(
                    bc[:], dstage[:].partition_broadcast(128))
                bsrc = bc
                nc.vector.tensor_mul(
                    ctxn[t][sp][0:64, half * 512:(half + 1) * 512],
                    pc[0:64, 0:512], bsrc[0:64, 0:512])
                nc.vector.tensor_mul(
                    ctxn[t][sp][64:128, half * 512:(half + 1) * 512],
                    pc[0:64, 512:1024], bsrc[64:128, 512:1024])

            # ---- program: front ------------------------------------------
            qk_group(0, 0)
            v_gen((0, 1))
            v_gen((2, 3))

            # filler: rest of f0 proj, v, f1 proj (order matches attn needs)
            for sq in range(1, SQ):
                filler.append((("qk", 0, sq), 1.8,
                               (lambda s: lambda: qk_group(s, 0))(sq)))
                filler.append((("v", sq), 2.1,
                               (lambda s: lambda: v_gen((4 * s,
                                                         4 * s + 1)))(sq)))
                filler.append((("v", sq), 2.1,
                               (lambda s: lambda: v_gen((4 * s + 2,
                                                         4 * s + 3)))(sq)))
            for sq in range(SQ):
                filler.append((("qk", 1, sq), 1.8,
                               (lambda s: lambda: qk_group(s, 1))(sq)))

            attn_j(0, 0)
            attn_j(0, 1)
            attn_j(0, 2)
            attn_j(0, 3)

            attn_j(1, 0)
            attn_j(1, 1)
            flush()
            # out-proj for s 0:1024 (ctxn sp0 of both pairs now final)
            for sh in range(2):
                for e in range(DT):
                    filler.append(
                        (("op", sh, e), 0.9,
                         (lambda a, b: lambda: outproj_group(a, 0, b))(e, sh)))
            attn_j(1, 2)
            # out-proj s 1024:1536 (sh0 of sp1) becomes legal after (1,2)
            attn_j(1, 3)
            flush()
            for e in range(DT):
                outproj_group(e, 1, 0)
            for e in range(DT):
                outproj_group(e, 1, 1, ob_act=True)

    nc.compile()
    return nc


_NC_CACHE = {}


def _get_nc():
    if "nc" not in _NC_CACHE:
        _NC_CACHE["nc"] = _build()
    return _NC_CACHE["nc"]


def _x_pre(xT):
    # [D, S] -> [sq*128+p, d*512+s] partition-major slabs
    return np.ascontiguousarray(
        xT.reshape(DT, 128, SQ, 512).transpose(2, 1, 0, 3)
        .reshape(SQ * 128, DT * 512)).astype(ml_dtypes.bfloat16)


def _w_pre(wT):
    # [D, F'] -> [p, d*F'+f]
    fw = wT.shape[1]
    return np.ascontiguousarray(
        wT.reshape(DT, 128, fw).transpose(1, 0, 2)
        .reshape(128, DT * fw)).astype(ml_dtypes.bfloat16)


def _in_maps(q, k, v, wq, bq, wk, bk, wv, bv, wo):
    maps = []
    xT = {}
    for b in range(B):
        xT[b] = tuple(_x_pre(x[b].T) for x in (q, k, v))
    per_g = {}
    for g in range(HG):
        sl = slice(g * F, (g + 1) * F)
        # interleave v weights/bias with the denominator ones-column per head
        wv_aug = np.zeros((D, FA), np.float32)
        bv_aug = np.zeros((FA,), np.float32)
        wv_sl = wv[sl, :]
        bv_sl = bv[sl]
        for h in range(HG):
            wv_aug[:, h * 65:h * 65 + 64] = wv_sl[h * 64:(h + 1) * 64, :].T
            bv_aug[h * 65:h * 65 + 64] = bv_sl[h * 64:(h + 1) * 64]
            bv_aug[h * 65 + 64] = 1.0
        per_g[g] = dict(
            wqT=_w_pre(wq[sl, :].T),
            wkT=_w_pre(wk[sl, :].T),
            wvT=_w_pre(wv_aug),
            woT=np.ascontiguousarray(
                wo[:, sl].T.reshape(FT, 128, D).transpose(1, 0, 2)
                .reshape(128, FT * D)).astype(ml_dtypes.bfloat16),
            bq8=np.ascontiguousarray((bq[sl] / 8.0).reshape(FT, 128).T),
            bk=np.ascontiguousarray(bk[sl].reshape(FT, 128).T),
            bv260=np.ascontiguousarray(np.broadcast_to(bv_aug, (128, FA))),
        )
    for c in range(N_CORES):
        b, g = c // HG, c % HG
        m = dict(xqT=xT[b][0], xkT=xT[b][1], xvT=xT[b][2])
        m.update(per_g[g])
        maps.append(m)
    return maps


def run(inputs, trace=False, tmpdir=None):
    nc = _get_nc()
    q = np.asarray(inputs["q"], np.float32)
    k = np.asarray(inputs["k"], np.float32)
    v = np.asarray(inputs["v"], np.float32)
    maps = _in_maps(q, k, v,
                    np.asarray(inputs["wq"], np.float32),
                    np.asarray(inputs["bq"], np.float32),
                    np.asarray(inputs["wk"], np.float32),
                    np.asarray(inputs["bk"], np.float32),
                    np.asarray(inputs["wv"], np.float32),
                    np.asarray(inputs["bv"], np.float32),
                    np.asarray(inputs["wo"], np.float32))
    kwargs = {}
    if trace:
        kwargs = dict(trace=True, tmpdir=tmpdir)
    res = bass_utils.run_bass_kernel_spmd(
        nc, maps, core_ids=list(range(N_CORES)), **kwargs)
    bo = np.asarray(inputs["bo"], np.float32)
    out = np.empty((B, S, D), np.float32)
    for b in range(B):
        acc = res.results[4 * b]["outT"].astype(np.float32)
        for g in range(1, HG):
            acc += res.results[4 * b + g]["outT"].astype(np.float32)
        out[b] = acc.T + bo
    return out, res


def kernel(**inputs):
    out, _ = run(inputs)
    return out


# revision 25
# speedup vs baseline: 1.1622x; 1.1622x over previous
"""Multi-head attention (B=2, S=2048, D=1024, H=16, causal) on 8 TRN2 NeuronCores.

Sharding: core c -> (batch b = c//4, head-group g = c%4, heads 4g..4g+3).
Each core computes Q/K/V projections for its 4 heads, causal attention, and
a partial output projection (its 256 d_model columns of ctx @ wo.T).  Host
sums the 4 partials per batch and adds bo.

v2 restructure vs baseline: the kernel is ACT(exp)-limited (~92us of exp),
so everything is scheduled to hide under the exp stream:
 - inputs DMA'd in s-quarter slabs so attention pair 0 starts ~17us in
   (f-tile-0 q/k projections + v quarter 0 only gate the first q-chunk);
 - f-tile-1 projections, remaining v projection, and output-projection
   matmuls are "filler" units pumped into PE slack inside the attention
   k-tile loop (PE ~0.65us/kt vs ACT 1.15us/kt);
 - softmax denominators via DVE reciprocal_approx_fast + DMA partition
   broadcast (pipelined per q-chunk; PE ones-matmul broadcast on the last
   chunk to cut the tail);
 - q/k projection drains on DVE (tensor_scalar w/ bias), output copies
   split DVE/ACT so ACT does (almost) only exp.
PSUM: sc tag [128,1024] bufs=2 (scores double-buffer) + pc tag [128,1024]
bufs=2 (ctx accumulator + one flex slot for proj/v/out-proj groups).
"""

import sys
from collections import deque

for _p in ("/opt/trn_rl_repo",):
    if _p not in sys.path:
        sys.path.insert(0, _p)

import ml_dtypes
import numpy as np

import concourse.bass as bass
import concourse.mybir as mybir
import concourse.tile as tile
from concourse import bacc, bass_utils

F32 = mybir.dt.float32
BF16 = mybir.dt.bfloat16
AF = mybir.ActivationFunctionType
ALU = mybir.AluOpType

N_CORES = 8
B, S, D, H = 2, 2048, 1024, 16
HG = 4              # heads per core
DK = 64             # head dim
F = HG * DK         # 256 features per core
FA = HG * (DK + 1)  # 260: v features + per-head denominator ones-column
DT = D // 128       # 8 d-chunks
FT = F // 128       # 2 f-tiles (head pairs)
SQ = 4              # s-quarters of 512


def _build():
    nc = bacc.Bacc("TRN2", target_bir_lowering=False, debug=False,
                   num_devices=N_CORES)

    def din(name, shape, dt=F32):
        return nc.dram_tensor(name, shape, dt, kind="ExternalInput").ap()

    # host pre-arranges everything partition-major so every DMA is a flat
    # 2D slab (3D fan-out DMAs race on their shared completion semaphore)
    xqT = din("xqT", (SQ * 128, DT * 512), BF16)  # [sq*128+p, d*512+s]
    xkT = din("xkT", (SQ * 128, DT * 512), BF16)
    xvT = din("xvT", (SQ * 128, DT * 512), BF16)
    wqT = din("wqT", (128, DT * F), BF16)         # [p, d*256+f]
    wkT = din("wkT", (128, DT * F), BF16)
    wvT = din("wvT", (128, DT * FA), BF16)        # interleaved ones-columns
    woT = din("woT", (128, FT * D), BF16)         # [p, t*1024+e]
    bq8 = din("bq8", (128, FT))
    bk = din("bk", (128, FT))
    bv260 = din("bv260", (128, FA))  # bv broadcast, 1.0 at ones-columns
    outT = nc.dram_tensor("outT", (D, S), BF16, kind="ExternalOutput").ap()

    with tile.TileContext(nc) as tc:
        with (
            tc.tile_pool(name="const", bufs=1) as cp,
            tc.tile_pool(name="data", bufs=1) as dp,
            tc.tile_pool(name="io", bufs=1) as iop,
            tc.tile_pool(name="dr", bufs=1, space="DRAM") as drp,
            tc.tile_pool(name="pp", bufs=2, space="PSUM") as pp,
        ):
            def psum_sc():
                return pp.tile([128, 1024], F32, name="sc", bufs=2)

            def psum_pc():
                return pp.tile([128, 1024], F32, name="pc", bufs=2)

            # ---- DMA issues: weights FIRST (small; v-proj matmuls sit at
            # the head of the in-order PE queue and must not wait 30us for
            # wv behind the input slabs), then input s-quarter slabs ------
            xq_t = [dp.tile([128, DT * 512], BF16, name=f"xq{sq}")
                    for sq in range(SQ)]
            xk_t = [dp.tile([128, DT * 512], BF16, name=f"xk{sq}")
                    for sq in range(SQ)]
            xv_t = [dp.tile([128, DT * 512], BF16, name=f"xv{sq}")
                    for sq in range(SQ)]
            wq_sb = cp.tile([128, DT * F], BF16, name="wq")
            wk_sb = cp.tile([128, DT * F], BF16, name="wk")
            wv_sb = cp.tile([128, DT * FA], BF16, name="wv")
            wo_sb = cp.tile([128, FT * D], BF16, name="wo")
            bq8_t = cp.tile([128, FT], F32, name="bq8")
            bk_t = cp.tile([128, FT], F32, name="bkt")
            bv_t = cp.tile([128, FA], F32, name="bv")
            nc.sync.dma_start(wq_sb[:], wqT[:])
            nc.sync.dma_start(wk_sb[:], wkT[:])
            nc.sync.dma_start(wv_sb[:], wvT[:])
            nc.sync.dma_start(bq8_t[:], bq8[:])
            nc.sync.dma_start(bk_t[:], bk[:])
            nc.sync.dma_start(bv_t[:], bv260[:])
            for sq in range(SQ):
                for xt, src in ((xq_t, xqT), (xk_t, xkT), (xv_t, xvT)):
                    nc.sync.dma_start(
                        xt[sq][:], src[sq * 128:(sq + 1) * 128, :])
            nc.sync.dma_start(wo_sb[:], woT[:])

            # ---- PE warmup (HAM un-throttle) on a DVE-memset tile --------
            wmup = cp.tile([128, 512], BF16, name="wmup")
            nc.vector.memset(wmup[:], 0.0)
            pw = psum_sc()
            for i in range(12):
                nc.tensor.matmul(pw[:, 0:512], wmup[:, 0:128], wmup[:],
                                 start=(i == 0), stop=True,
                                 skip_group_check=True)

            # prepay the exp ACT_TABLE_LOAD (~2.7us) while DMAs stream
            escr = cp.tile([1, 16], F32, name="escr")
            nc.scalar.activation(escr[:], wmup[0:1, 0:16], AF.Exp)

            # tri01[p, y] = 1 if y >= p else 0 (diagonal-block causal mask)
            tri = cp.tile([128, 128], BF16, name="tri")
            nc.gpsimd.memset(tri[:], 1.0)
            nc.gpsimd.affine_select(
                out=tri[:], in_=tri[:], compare_op=ALU.is_ge,
                fill=0.0, base=0, pattern=[[1, 128]], channel_multiplier=-1)
            # absorb gpsimd tensor_tensor dispatch warmup off critical path
            gwarm = cp.tile([128, 128], BF16, name="gwarm")
            nc.gpsimd.tensor_tensor(gwarm[:], tri[:], tri[:], op=ALU.mult)

            # ---- persistent per-core tensors -----------------------------
            # qpT/kpT[t][sp]: [f 128, s 1024];  ctxn[t][sp] same layout
            qpT = [[dp.tile([128, 1024], BF16, name=f"qpT{t}{sp}")
                    for sp in range(2)] for t in range(FT)]
            kpT = [[dp.tile([128, 1024], BF16, name=f"kpT{t}{sp}")
                    for sp in range(2)] for t in range(FT)]
            ctxn = [[dp.tile([128, 1024], BF16, name=f"ctxn{t}{sp}")
                     for sp in range(2)] for t in range(FT)]
            vp = [dp.tile([128, FA], BF16, name=f"vp{st}") for st in range(16)]

            # ---- building blocks -----------------------------------------
            def qk_group(sq, t):
                """f-tile t of q and k projections for s-quarter sq.
                One psum slot: q in cols 0:512, k in 512:1024."""
                ps = psum_pc() if t == 1 else psum_sc()
                for d in range(DT):
                    nc.tensor.matmul(
                        ps[:, 0:512],
                        wq_sb[:, d * F + t * 128: d * F + (t + 1) * 128],
                        xq_t[sq][:, d * 512:(d + 1) * 512],
                        start=(d == 0), stop=(d == DT - 1))
                for d in range(DT):
                    nc.tensor.matmul(
                        ps[:, 512:1024],
                        wk_sb[:, d * F + t * 128: d * F + (t + 1) * 128],
                        xk_t[sq][:, d * 512:(d + 1) * 512],
                        start=(d == 0), stop=(d == DT - 1))
                sp, half = sq // 2, sq % 2
                nc.vector.tensor_scalar(
                    qpT[t][sp][:, half * 512:(half + 1) * 512],
                    ps[:, 0:512], 0.125, bq8_t[:, t:t + 1],
                    op0=ALU.mult, op1=ALU.add)
                nc.vector.tensor_scalar_add(
                    kpT[t][sp][:, half * 512:(half + 1) * 512],
                    ps[:, 512:1024], bk_t[:, t:t + 1])

            def v_gen(s8_pair):
                """v projection for two s8 tiles (128 seq positions each)."""
                ps = psum_pc()
                for d in range(DT):
                    for i, s8 in enumerate(s8_pair):
                        sq, s8i = s8 // 4, s8 % 4
                        nc.tensor.matmul(
                            ps[:, i * 512:i * 512 + FA],
                            xv_t[sq][:, d * 512 + s8i * 128:
                                     d * 512 + (s8i + 1) * 128],
                            wv_sb[:, d * FA:(d + 1) * FA],
                            start=(d == 0), stop=(d == DT - 1))
                for i, s8 in enumerate(s8_pair):
                    nc.vector.tensor_add(
                        vp[s8][:], ps[:, i * 512:i * 512 + FA], bv_t[:])

            def outproj_group(e, sp, sh, ob_act=False):
                """out rows e*128.., s-cols sp*1024+sh*512 .. +512."""
                ps = psum_pc()
                for t in range(FT):
                    nc.tensor.matmul(
                        ps[:, 0:512],
                        wo_sb[:, t * D + e * 128: t * D + (e + 1) * 128],
                        ctxn[t][sp][:, sh * 512:(sh + 1) * 512],
                        start=(t == 0), stop=(t == FT - 1))
                ob = iop.tile([128, 512], BF16, name="ob", bufs=4)
                if ob_act:
                    nc.scalar.copy(ob[:], ps[:, 0:512])
                else:
                    nc.vector.tensor_copy(ob[:], ps[:, 0:512])
                nc.sync.dma_start(
                    outT[e * 128:(e + 1) * 128,
                         sp * 1024 + sh * 512: sp * 1024 + (sh + 1) * 512],
                    ob[:])

            # ---- filler pump: PE work hidden in ACT-limited attn windows.
            # Tile deps are trace-order based (a read traced before its
            # producer's write gets NO dependency), so attn_j must require()
            # its producers before tracing consumers; pump() spends leftover
            # slack on whatever is next.
            filler = deque()   # (key, cost_us, emit_fn)
            _budget = [0.0]

            def pump(budget_us):
                _budget[0] += budget_us
                while filler and filler[0][1] <= _budget[0]:
                    key, cost, fn = filler.popleft()
                    fn()
                    _budget[0] -= cost

            def require(keys):
                keep, run = deque(), []
                for item in filler:
                    (run if item[0] in keys else keep).append(item)
                filler.clear()
                filler.extend(keep)
                for _, _, fn in run:
                    fn()

            def flush():
                while filler:
                    filler.popleft()[2]()
                _budget[0] = 0.0

            # ---- attention -----------------------------------------------
            def attn_j(hp, j):
                t = hp
                require({("qk", hp, s) for s in range(j + 1)}
                        | {("v", s) for s in range(j + 1)})
                q0 = j * 512
                sp, half = j // 2, j % 2
                pc = psum_pc()            # ctx: h0 in 0:512, h1 in 512:1024
                started = [False, False]
                kts = list(range(4 * j + 4))  # ascending: ctx group starts full-width

                def ctx_mm(kt, ex, c0, last):
                    w = 512 - c0
                    for hh in range(2):
                        nc.tensor.matmul(
                            pc[0:65, hh * 512 + c0: hh * 512 + 512],
                            vp[kt][:, 65 * (2 * hp + hh):
                                   65 * (2 * hp + hh) + 65],
                            ex[:, hh * 512: hh * 512 + w],
                            start=not started[hh], stop=last)
                        started[hh] = True

                pending = []
                for n, kt in enumerate(kts):
                    c0 = max(0, 128 * kt - q0)
                    w = 512 - c0
                    psc = psum_sc()
                    for hh, off in ((0, 0), (1, 64)):
                        nc.tensor.matmul(
                            psc[:, hh * 512: hh * 512 + w],
                            kpT[t][kt // 8][off:off + 64,
                                            (kt % 8) * 128:(kt % 8 + 1) * 128],
                            qpT[t][sp][off:off + 64,
                                       half * 512 + c0: half * 512 + 512],
                            start=True, stop=True)
                    ex = iop.tile([128, 1024], BF16, name="ex", bufs=6)
                    if w == 512:
                        nc.scalar.activation(ex[:], psc[:], AF.Exp)
                    else:
                        v2 = psc[:].rearrange("p (b c) -> p b c",
                                              c=512)[:, :, 0:w]
                        e2 = ex[:].rearrange("p (b c) -> p b c",
                                             c=512)[:, :, 0:w]
                        nc.scalar.activation(e2, v2, AF.Exp)
                    if 128 * kt >= q0:   # diagonal block: triangular mask
                        nc.gpsimd.tensor_tensor(
                            ex[:, 0:128], ex[:, 0:128], tri[:], op=ALU.mult)
                        nc.gpsimd.tensor_tensor(
                            ex[:, 512:640], ex[:, 512:640], tri[:],
                            op=ALU.mult)
                    pending.append((kt, ex, c0))
                    if len(pending) > 1:
                        ctx_mm(*pending.pop(0), last=False)
                    pump(0.6)
                while pending:
                    ctx_mm(*pending.pop(0), last=(len(pending) == 0))

                # drain ctx+den rows to SBUF in one copy (frees the pc slot
                # in ~1us), then normalize off-PSUM: den -> DRAM -> [128,8]
                # layout -> fast 128-lane reciprocal -> DRAM -> stride-0
                # partition broadcast -> multiply
                cu = iop.tile([65, 1024], F32, name="cu", bufs=3)
                nc.vector.tensor_copy(cu[:], pc[0:65, :])
                dstage = drp.tile([1, 1024], F32, name="dstage", bufs=2)
                nc.sync.dma_start(dstage[:], cu[64:65, :])
                d8 = iop.tile([128, 8], F32, name="d8", bufs=2)
                nc.sync.dma_start(
                    d8[:],
                    dstage[:].rearrange("o (p c) -> (o p) c", p=128))
                r8 = iop.tile([128, 8], F32, name="r8", bufs=2)
                nc.vector.reciprocal(r8[:], d8[:])
                rstage = drp.tile([1, 1024], F32, name="rstage", bufs=2)
                nc.sync.dma_start(
                    rstage[:].rearrange("o (p c) -> (o p) c", p=128),
                    r8[:])
                bc = iop.tile([128, 1024], F32, name="bc", bufs=3)
                nc.sync.dma_start(
                    bc[:], rstage[:].partition_broadcast(128))
                nc.vector.tensor_mul(
                    ctxn[t][sp][0:64, half * 512:(half + 1) * 512],
                    cu[0:64, 0:512], bc[0:64, 0:512])
                nc.vector.tensor_mul(
                    ctxn[t][sp][64:128, half * 512:(half + 1) * 512],
                    cu[0:64, 512:1024], bc[0:64, 512:1024])

            # ---- program: front ------------------------------------------
            qk_group(0, 0)
            v_gen((0, 1))
            v_gen((2, 3))

            # filler: rest of f0 proj, v, f1 proj (order matches attn needs)
            for sq in range(1, SQ):
                filler.append((("qk", 0, sq), 1.8,
                               (lambda s: lambda: qk_group(s, 0))(sq)))
                filler.append((("v", sq), 2.1,
                               (lambda s: lambda: v_gen((4 * s,
                                                         4 * s + 1)))(sq)))
                filler.append((("v", sq), 2.1,
                               (lambda s: lambda: v_gen((4 * s + 2,
                                                         4 * s + 3)))(sq)))
            for sq in range(SQ):
                filler.append((("qk", 1, sq), 1.8,
                               (lambda s: lambda: qk_group(s, 1))(sq)))

            attn_j(0, 0)
            attn_j(0, 1)
            attn_j(0, 2)
            attn_j(0, 3)

            attn_j(1, 0)
            attn_j(1, 1)
            flush()
            # out-proj for s 0:1024 (ctxn sp0 of both pairs now final)
            for sh in range(2):
                for e in range(DT):
                    filler.append(
                        (("op", sh, e), 0.9,
                         (lambda a, b: lambda: outproj_group(a, 0, b))(e, sh)))
            attn_j(1, 2)
            # out-proj s 1024:1536 (sh0 of sp1) legal after (1,2): pump it
            # inside attn(1,3)'s window
            for e in range(DT):
                filler.append(
                    (("op", 2, e), 0.9,
                     (lambda a: lambda: outproj_group(a, 1, 0))(e)))
            attn_j(1, 3)
            flush()
            for e in range(DT):
                outproj_group(e, 1, 1, ob_act=True)

    nc.compile()
    return nc


_NC_CACHE = {}


def _get_nc():
    if "nc" not in _NC_CACHE:
        _NC_CACHE["nc"] = _build()
    return _NC_CACHE["nc"]


def _x_pre(xT):
    # [D, S] -> [sq*128+p, d*512+s] partition-major slabs
    return np.ascontiguousarray(
        xT.reshape(DT, 128, SQ, 512).transpose(2, 1, 0, 3)
        .reshape(SQ * 128, DT * 512)).astype(ml_dtypes.bfloat16)


def _w_pre(wT):
    # [D, F'] -> [p, d*F'+f]
    fw = wT.shape[1]
    return np.ascontiguousarray(
        wT.reshape(DT, 128, fw).transpose(1, 0, 2)
        .reshape(128, DT * fw)).astype(ml_dtypes.bfloat16)


def _in_maps(q, k, v, wq, bq, wk, bk, wv, bv, wo):
    maps = []
    xT = {}
    for b in range(B):
        xT[b] = tuple(_x_pre(x[b].T) for x in (q, k, v))
    per_g = {}
    for g in range(HG):
        sl = slice(g * F, (g + 1) * F)
        # interleave v weights/bias with the denominator ones-column per head
        wv_aug = np.zeros((D, FA), np.float32)
        bv_aug = np.zeros((FA,), np.float32)
        wv_sl = wv[sl, :]
        bv_sl = bv[sl]
        for h in range(HG):
            wv_aug[:, h * 65:h * 65 + 64] = wv_sl[h * 64:(h + 1) * 64, :].T
            bv_aug[h * 65:h * 65 + 64] = bv_sl[h * 64:(h + 1) * 64]
            bv_aug[h * 65 + 64] = 1.0
        per_g[g] = dict(
            wqT=_w_pre(wq[sl, :].T),
            wkT=_w_pre(wk[sl, :].T),
            wvT=_w_pre(wv_aug),
            woT=np.ascontiguousarray(
                wo[:, sl].T.reshape(FT, 128, D).transpose(1, 0, 2)
                .reshape(128, FT * D)).astype(ml_dtypes.bfloat16),
            bq8=np.ascontiguousarray((bq[sl] / 8.0).reshape(FT, 128).T),
            bk=np.ascontiguousarray(bk[sl].reshape(FT, 128).T),
            bv260=np.ascontiguousarray(np.broadcast_to(bv_aug, (128, FA))),
        )
    for c in range(N_CORES):
        b, g = c // HG, c % HG
        m = dict(xqT=xT[b][0], xkT=xT[b][1], xvT=xT[b][2])
        m.update(per_g[g])
        maps.append(m)
    return maps


def run(inputs, trace=False, tmpdir=None):
    nc = _get_nc()
    q = np.asarray(inputs["q"], np.float32)
    k = np.asarray(inputs["k"], np.float32)
    v = np.asarray(inputs["v"], np.float32)
    maps = _in_maps(q, k, v,
                    np.asarray(inputs["wq"], np.float32),
                    np.asarray(inputs["bq"], np.float32),
                    np.asarray(inputs["wk"], np.float32),
                    np.asarray(inputs["bk"], np.float32),
                    np.asarray(inputs["wv"], np.float32),
                    np.asarray(inputs["bv"], np.float32),
                    np.asarray(inputs["wo"], np.float32))
    kwargs = {}
    if trace:
        kwargs = dict(trace=True, tmpdir=tmpdir)
    res = bass_utils.run_bass_kernel_spmd(
        nc, maps, core_ids=list(range(N_CORES)), **kwargs)
    bo = np.asarray(inputs["bo"], np.float32)
    out = np.empty((B, S, D), np.float32)
    for b in range(B):
        acc = res.results[4 * b]["outT"].astype(np.float32)
        for g in range(1, HG):
            acc += res.results[4 * b + g]["outT"].astype(np.float32)
        out[b] = acc.T + bo
    return out, res


def kernel(**inputs):
    out, _ = run(inputs)
    return out
